# revision 7
# baseline (speedup 1.0000x reference)
"""Trainium2 Bass kernel for nn_CLARM_56693568307877.

Computes, for feature sets A [64,640,14,14] and B [128,640,14,14] and a QKV
projection W [240,640]:
    q,k,v = split(x^T W^T); S = q_b k_a^T / sqrt(80); P = softmax(S)
    rec = P v_a;  sim[b,a] = -||v_b - rec||^2_F
Output [128, 64] fp32.

Sharding: data-parallel over the b batch (16 per core x 8 cores);
features_a / W replicated. bf16 device-side with fp32 accumulation.

v2 design (per core: B=16, A=64, N=M=196, D=80), engine-balanced around the
irreducible ACT exp cost (~0.41-0.46 ms):
  phase 1: QKV on PE. q_b, k_a d-major; v_b and v_a produced directly in
           transposed layouts via stationary-x matmuls (v_b n-major 98-chunks,
           v_a m-major for the mm2 stationary-free moving operand).
  per 4-b subgroup:
    mm1   PE : S^T = k_a-chunk^T q_b -> 4 psum banks [128(m), 392]
    exp   ACT: one [128,4,392] activation when the 4 banks are contiguous
    mm2   PE : stationary = E 98-col chunks, moving = v_aug [m, 81]
               -> U^T n-major [98, 8-chunk, 81] in 2 psum banks; col 80 is the
               softmax denominator w (ones column of v_aug)
    tail  DVE: one reciprocal (4-dim AP over both banks' col 80) -> winv bf16,
               one rec TT (U^T * winv zero-stride-broadcast along d),
               one sub TT (rec - v_b n-major), one sq TT
          Pool: per-pair XYZWC reduce -> -sim scalars into a wave row
  per 4-a wave: negate [1,64] on DVE, DMA to simo.
No per-subgroup DMAs: the old denominator gather / reciprocal broadcast and
the DmaTranspose of v_a are gone (~740 fewer HWDGE slots).

Walrus notes: single semaphore wait per instruction (_split_multi_waits),
cross-lane reduce rejects negate=True (negate done on DVE), tensor_scalar pow
rejected (square via TT mult).
"""

import numpy as np
import ml_dtypes

import concourse.bass as bass
import concourse.tile as tile
from concourse import mybir
from concourse.bass_utils import run_bass_kernel_spmd

BF16 = mybir.dt.bfloat16
F32 = mybir.dt.float32

NCORES = 8
A_FULL = 64
B_FULL = 128
HID = 640
KC = HID // 128  # 5
N = 196          # tokens (14*14)
D = 80           # inner dim
MPAD = 256       # m padded to 2*128 for clean mm1 stationary chunks
NCH = 8          # 98-col chunks per subgroup (4 b * 196 n = 784 = 8*98)
CW = 98          # chunk width
SCALE = 1.0 / np.sqrt(D)

_PROGRAM_CACHE = {}
DEBUG_TAPS = False


def _build(Asz, Bsz):
    """Emit the Bass program for one core handling Bsz b's x Asz a's."""
    assert Bsz % 4 == 0 and Asz % 4 == 0
    NSG = Bsz // 4            # 4-b subgroups per a
    PW = 4 * Bsz              # pairs per 4-a wave

    nc = bass.Bass("TRN2", debug=False)
    fa = nc.dram_tensor("fa", [Asz, KC, 128, N], BF16, kind="ExternalInput")
    fb = nc.dram_tensor("fb", [Bsz, KC, 128, N], BF16, kind="ExternalInput")
    wt = nc.dram_tensor("wt", [KC, 128, 240], BF16, kind="ExternalInput")
    simo = nc.dram_tensor("sim", [Asz, Bsz], F32, kind="ExternalOutput")
    if DEBUG_TAPS:
        dbg_w = nc.dram_tensor("dbg_w", [CW, NCH], F32, kind="ExternalOutput")
        dbg_rec = nc.dram_tensor("dbg_rec", [CW, NCH, D], F32, kind="ExternalOutput")
        dbg_d2 = nc.dram_tensor("dbg_d2", [CW, NCH, D], F32, kind="ExternalOutput")
        dbg_e = nc.dram_tensor("dbg_e", [128, 4, 392], F32, kind="ExternalOutput")

    Exp = mybir.ActivationFunctionType.Exp
    mult = mybir.AluOpType.mult
    sub = mybir.AluOpType.subtract

    with tile.TileContext(nc) as tc:
        with (
            tc.tile_pool(name="const", bufs=1) as cpool,
            tc.tile_pool(name="ring", bufs=1, space="PSUM") as rpool,
            tc.tile_pool(name="x", bufs=3) as x_pool,
            tc.tile_pool(name="e", bufs=3) as e_pool,
            tc.tile_pool(name="w", bufs=6) as w_pool,
            tc.tile_pool(name="rec", bufs=4) as rec_pool,
            tc.tile_pool(name="d", bufs=4) as d_pool,
            tc.tile_pool(name="wave", bufs=2) as wv_pool,
        ):
            wt_sb = cpool.tile([128, KC, 240], BF16, tag="wt")
            kT_all = cpool.tile([128, Asz, MPAD], BF16, tag="kT")
            vaug = cpool.tile([128, Asz, 2, 81], BF16, tag="vaug")
            qT_all = cpool.tile([128, Bsz, N], BF16, tag="qT")
            vbn_all = cpool.tile([CW, Bsz, 2, D], BF16, tag="vbn")
            ring = rpool.tile([128, 8, 512], F32, tag="ring")

            # one-time init
            nc.sync.dma_start(wt_sb, wt.ap().rearrange("k p c -> p k c"))
            nc.gpsimd.memset(kT_all[:], 0.0)
            nc.gpsimd.memset(qT_all[:], 0.0)
            nc.gpsimd.memset(vaug[:], 0.0)
            # ones column of v_aug: rows = valid m per chunk (128 / 68)
            nc.gpsimd.memset(vaug[0:128, :, 0, 80:81], 1.0)
            nc.gpsimd.memset(vaug[0:68, :, 1, 80:81], 1.0)

            rp = [0]

            def rslot(k=1):
                s = rp[0] % 8
                rp[0] += k
                return s

            # ---- phase 1b: q (d-major) + v_b (n-major) for all b ----
            for b in range(Bsz):
                xt = x_pool.tile([128, KC, N], BF16, tag="x")
                nc.sync.dma_start(xt, fb[b].rearrange("k p n -> p k n"))
                sq_ = rslot()
                psq = ring[0:80, sq_, 0:N]
                for kc in range(KC):
                    nc.tensor.matmul(
                        psq, wt_sb[:, kc, 0:80], xt[:, kc, :],
                        start=(kc == 0), stop=(kc == KC - 1),
                    )
                nc.scalar.copy(qT_all[0:80, b, :], psq)
                sv = rslot()
                for h in range(2):
                    psv = ring[0:CW, sv, 80 * h:80 * h + 80]
                    for kc in range(KC):
                        nc.tensor.matmul(
                            psv,
                            xt[:, kc, CW * h:CW * h + CW],
                            wt_sb[:, kc, 160:240],
                            start=(kc == 0), stop=(kc == KC - 1),
                        )
                nc.scalar.copy(
                    vbn_all[0:CW, b, :, :].rearrange("p h d -> p (h d)"),
                    ring[0:CW, sv, 0:160],
                )

            wave_a0 = 0
            simrow = None

            for a in range(Asz):
                if a % 4 == 0:
                    wave_a0 = a
                    simrow = wv_pool.tile([1, PW], F32, tag="sr")
                # ---- phase 1a for this a: k (d-major) + v_a (m-major) ----
                xt = x_pool.tile([128, KC, N], BF16, tag="x")
                nc.sync.dma_start(xt, fa[a].rearrange("k p n -> p k n"))
                sk = rslot()
                psk = ring[0:80, sk, 0:N]
                for kc in range(KC):
                    nc.tensor.matmul(
                        psk, wt_sb[:, kc, 80:160], xt[:, kc, :],
                        start=(kc == 0), stop=(kc == KC - 1),
                    )
                nc.vector.tensor_copy(kT_all[0:80, a, 0:N], psk)
                sv = rslot()
                for mc in range(2):
                    rows = 128 if mc == 0 else 68
                    psv = ring[0:rows, sv, 80 * mc:80 * mc + 80]
                    for kc in range(KC):
                        nc.tensor.matmul(
                            psv,
                            xt[:, kc, 128 * mc:128 * mc + rows],
                            wt_sb[:, kc, 160:240],
                            start=(kc == 0), stop=(kc == KC - 1),
                        )
                nc.vector.tensor_copy(
                    vaug[0:128, a, 0, 0:80], ring[0:128, sv, 0:80]
                )
                nc.vector.tensor_copy(
                    vaug[0:68, a, 1, 0:80], ring[0:68, sv, 80:160]
                )

                for sgb in range(NSG):
                    b0 = 4 * sgb
                    # ---- mm1: S^T chunks -> 4 slots ----
                    s_e = rslot(4)
                    for mc in range(2):
                        for ncx in range(2):
                            nc.tensor.matmul(
                                ring[:, (s_e + 2 * mc + ncx) % 8, 0:392],
                                kT_all[:, a, mc * 128:(mc + 1) * 128],
                                qT_all[:, b0 + 2 * ncx: b0 + 2 * ncx + 2, :],
                                start=True, stop=True,
                            )
                    # ---- exp ----
                    e = e_pool.tile([128, 4, 392], BF16, tag="e")
                    if s_e + 3 <= 7:
                        nc.scalar.activation(
                            e, ring[:, s_e:s_e + 4, 0:392], Exp
                        )
                    else:
                        nc.scalar.activation(
                            e[:, 0:2, :], ring[:, s_e:s_e + 2, 0:392], Exp
                        )
                        nc.scalar.activation(
                            e[:, 2:4, :], ring[:, 0:2, 0:392], Exp
                        )
                    # ---- mm2': U^T n-major [98, 8, 81] in 2 banks ----
                    u0 = rslot(2)
                    assert u0 % 2 == 0, u0
                    e_flat = e.rearrange("p c n -> p (c n)")
                    for c in range(NCH):
                        for kc in range(2):
                            rows = 128 if kc == 0 else 68
                            nc.tensor.matmul(
                                ring[0:CW, u0 + c // 4, 81 * (c % 4):81 * (c % 4) + 81],
                                e_flat[0:rows, 784 * kc + CW * c: 784 * kc + CW * c + CW],
                                vaug[0:rows, a, kc, :],
                                start=(kc == 0), stop=(kc == 1),
                            )
                    # ---- tail (n-major, engine-local) ----
                    winv = w_pool.tile([CW, NCH], F32, tag="winv")
                    wsrc = bass.AP(
                        ring.tensor,
                        ring.offset + (u0 * 512 + 80),
                        [[ring.ap[0][0], CW], [512, 2], [81, 4]],
                    )
                    nc.vector.reciprocal(winv, wsrc)
                    recs = rec_pool.tile([CW, NCH, D], BF16, tag="rec")
                    usrc = bass.AP(
                        ring.tensor,
                        ring.offset + u0 * 512,
                        [[ring.ap[0][0], CW], [512, 2], [81, 4], [1, D]],
                    )
                    wbc = bass.AP(
                        winv.tensor, winv.offset,
                        [[winv.ap[0][0], CW], [4, 2], [1, 4], [0, D]],
                    )
                    nc.vector.tensor_tensor(
                        recs.rearrange("p (u c) d -> p u c d", u=2), usrc, wbc,
                        op=mult,
                    )
                    dt_ = d_pool.tile([CW, NCH, D], BF16, tag="d")
                    nc.vector.tensor_tensor(
                        dt_, recs,
                        vbn_all[:, b0:b0 + 4, :, :].rearrange("p b h d -> p (b h) d"),
                        op=sub,
                    )
                    d2 = d_pool.tile([CW, NCH, D], BF16, tag="d2")
                    nc.vector.tensor_tensor(d2, dt_, dt_, op=mult)
                    if DEBUG_TAPS and a == 0 and sgb == 0:
                        wf = w_pool.tile([CW, NCH], F32, tag="wf")
                        nc.vector.tensor_copy(
                            wf.rearrange("p (u c) -> p u c", u=2), wsrc
                        )
                        nc.sync.dma_start(dbg_w.ap(), wf)
                        rf = rec_pool.tile([CW, NCH, D], F32, tag="rf")
                        nc.vector.tensor_copy(rf, recs)
                        nc.sync.dma_start(dbg_rec.ap(), rf)
                        d2f = d_pool.tile([CW, NCH, D], F32, tag="d2f")
                        nc.vector.tensor_copy(d2f, d2)
                        nc.sync.dma_start(dbg_d2.ap(), d2f)
                        ef = e_pool.tile([128, 4, 392], F32, tag="ef")
                        nc.vector.tensor_copy(ef, e)
                        nc.sync.dma_start(dbg_e.ap(), ef)
                    col0 = (a % 4) * Bsz + b0
                    for j in range(4):
                        nc.gpsimd.tensor_reduce(
                            out=simrow[0:1, col0 + j:col0 + j + 1],
                            in_=d2[:, 2 * j:2 * j + 2, :],
                            axis=mybir.AxisListType.XYZWC,
                            op=mybir.AluOpType.add,
                        )

                if a % 4 == 3:
                    srn = wv_pool.tile([1, PW], F32, tag="srn")
                    nc.vector.tensor_scalar(
                        out=srn, in0=simrow, scalar1=-1.0, scalar2=None,
                        op0=mult,
                    )
                    nc.sync.dma_start(simo[wave_a0:wave_a0 + 4, :], srn[0:1, :])

    return nc


def _split_multi_waits(nc):
    """This walrus build accepts at most one semaphore wait per instruction;
    Tile emits several (incl. its tail drain). Hoist extra waits onto
    single-wait engine NoOps inserted just before the instruction."""
    cnt = 0
    for f in nc.m.functions:
        for bb in f.blocks:
            insts = list(bb.instructions)
            out = []
            changed = False
            for inst in insts:
                si = getattr(inst, "sync_info", None)
                ws = list(si.on_wait) if (si is not None and si.on_wait) else []
                if len(ws) > 1:
                    changed = True
                    for w in ws[:-1]:
                        cnt += 1
                        out.append(mybir.InstNoOp(
                            name=f"WSPLIT-{cnt}",
                            engine=inst.engine,
                            ins=[], outs=[],
                            sync_info=mybir.SyncInfo(on_wait=[w], on_update=[]),
                        ))
                    si.on_wait = [ws[-1]]
                    inst.sync_info = si
                out.append(inst)
            if changed:
                bb.instructions = out
    return nc


def _get_program(Asz, Bsz):
    key = (Asz, Bsz)
    if key not in _PROGRAM_CACHE:
        _PROGRAM_CACHE[key] = _split_multi_waits(_build(Asz, Bsz))
    return _PROGRAM_CACHE[key]


def _prep_inputs(features_a, features_b, W_qkv, Asz, Bsz, ncores):
    """Host-side: cast to bf16, fold the 1/sqrt(D) scale into Wq, reshape."""
    fa = features_a.reshape(Asz, HID, N).astype(ml_dtypes.bfloat16)
    fa = fa.reshape(Asz, KC, 128, N)
    wt = W_qkv.T.copy().astype(np.float32)   # [640, 240]
    wt[:, 0:D] *= SCALE
    wt = wt.astype(ml_dtypes.bfloat16).reshape(KC, 128, 240)
    fbs = []
    for c in range(ncores):
        fb = features_b[c * Bsz:(c + 1) * Bsz].reshape(Bsz, HID, N)
        fb = fb.astype(ml_dtypes.bfloat16).reshape(Bsz, KC, 128, N)
        fbs.append(fb)
    return fa, fbs, wt


def kernel(features_a, features_b, W_qkv):
    Asz = features_a.shape[0]
    Bfull = features_b.shape[0]
    ncores = NCORES
    Bsz = Bfull // ncores
    fa, fbs, wt = _prep_inputs(
        np.asarray(features_a), np.asarray(features_b), np.asarray(W_qkv),
        Asz, Bsz, ncores,
    )
    nc = _get_program(Asz, Bsz)
    in_maps = [{"fa": fa, "fb": fbs[c], "wt": wt} for c in range(ncores)]
    res = run_bass_kernel_spmd(nc, in_maps, core_ids=list(range(ncores)))
    out = np.concatenate([res.results[c]["sim"].T for c in range(ncores)], axis=0)
    return out.astype(np.float32)
